# revision 21
# baseline (speedup 1.0000x reference)
"""Trainium2 Bass kernel for 3-layer GAT + ODE stage (nn_GAT_37151467110676).

Strategy: nodes sharded by destination across 8 cores (2500 each); edge list
(with self-loops) sorted by dst and tiled per 128-dst tile, padded to a uniform
chunk count. Per 128-edge chunk the alpha-weighted one-hot matrix A[e, h*128+d]
is built with one fused tensor_scalar per head and TensorE contracts
X_src.T @ A into per-dst aggregates (plus softmax denominators); softmax
normalization is applied post-projection where the denominator is a
per-partition scalar. Node-feature tables for the next layer are exchanged with
an AllGather. The ODE stage is elementwise per node.
"""
import sys
sys.path.insert(0, '/opt/trn_rl_repo')
import numpy as np

N = 20000
NCORES = 8
NPC = N // NCORES          # 2500 nodes per core
P = 128
NT = (NPC + P - 1) // P    # 20 dst tiles per core (last has 68 rows)
BLK = NPC + 1              # 2501 rows per core block in gather tables (w/ sentinel)
NROWS = BLK * NCORES       # 20008
SENT = -30000.0
BN_EPS = 1e-5

TRACE = False
DEBUG = False
_CACHE = {}
_last_exec_ns = None
_last_results = None


# ----------------------------------------------------------------- host prep
def _prep(inputs):
    f32 = np.float32
    g = int(np.asarray(inputs["gene_index"]))
    x = np.asarray(inputs["x"], f32)
    ei = np.asarray(inputs["edge_index"], np.int64)
    W0 = np.asarray(inputs["W0"], f32); W1 = np.asarray(inputs["W1"], f32)
    W2 = np.asarray(inputs["W2"], f32)
    as0 = np.asarray(inputs["as0"], f32); ad0 = np.asarray(inputs["ad0"], f32)
    as1 = np.asarray(inputs["as1"], f32); ad1 = np.asarray(inputs["ad1"], f32)
    as2 = np.asarray(inputs["as2"], f32); ad2 = np.asarray(inputs["ad2"], f32)
    b0 = np.asarray(inputs["b0"], f32); b1 = np.asarray(inputs["b1"], f32)
    b2 = np.asarray(inputs["b2"], f32)
    kk = float(np.asarray(inputs["k"], f32)[g]); dd = float(np.asarray(inputs["d"], f32)[g])
    t0 = float(np.asarray(inputs["t0"], f32)[g]); u0 = float(np.asarray(inputs["u0"], f32)[g])
    t = np.asarray(inputs["t"], f32)

    # score projection vectors: ss[n,h] = x @ vs[:,h]
    def sv(W, a, H, C):
        Wr = W.reshape(H, C, -1)
        return np.einsum('hcj,hc->jh', Wr, a).astype(f32)
    v0s, v0d = sv(W0, as0, 4, 128), sv(W0, ad0, 4, 128)      # (2,4)
    v1s, v1d = sv(W1, as1, 4, 128), sv(W1, ad1, 4, 128)      # (128,4)
    v2s, v2d = sv(W2, as2, 1, 3), sv(W2, ad2, 1, 3)          # (128,1)

    # edges with self loops, sorted by dst
    src = np.concatenate([ei[0], np.arange(N, dtype=np.int64)])
    dst = np.concatenate([ei[1], np.arange(N, dtype=np.int64)])
    order = np.argsort(dst, kind='stable')
    src = src[order]; dst = dst[order]

    # tile boundaries: core c tile t covers dst [c*NPC + t*P, ...)
    bounds = []
    for c in range(NCORES):
        for tt_ in range(NT):
            lo = c * NPC + tt_ * P
            hi = min(lo + P, (c + 1) * NPC)
            bounds.append((lo, hi))
    los = np.array([b[0] for b in bounds])
    his = np.array([b[1] for b in bounds])
    e_lo = np.searchsorted(dst, los, side='left')
    e_hi = np.searchsorted(dst, his, side='left')
    cnt = e_hi - e_lo
    nchunk = int(np.ceil(cnt.max() / P))
    nidx = nchunk * P

    rid = (src // NPC) * BLK + (src % NPC)       # remapped src row ids
    rid_d = (dst // NPC) * BLK + (dst % NPC)     # remapped dst row ids

    xg = np.zeros((NCORES, NT, nidx), np.int64)
    sg = np.zeros((NCORES, NT, nidx), np.int64)
    dl = np.zeros((NCORES, NT, nidx), f32)
    for c in range(NCORES):
        for tt_ in range(NT):
            i = c * NT + tt_
            lo, hi, n_e = e_lo[i], e_hi[i], cnt[i]
            xg[c, tt_, :n_e] = rid[lo:hi]
            sg[c, tt_, :n_e] = rid_d[lo:hi]
            dl[c, tt_, :n_e] = (dst[lo:hi] - los[i]).astype(f32)
            xg[c, tt_, n_e:] = c * BLK + NPC              # sentinel row
            sg[c, tt_, n_e:] = rid_d[lo] if n_e > 0 else c * BLK
            dl[c, tt_, n_e:] = 0.0

    def wrap16(a):  # (NT, nidx) -> (128, NT, nidx//16) int16, idx i at [i%16, t, i//16]
        w = a.reshape(NT, nidx // 16, 16).transpose(2, 0, 1)
        return np.tile(w, (8, 1, 1)).astype(np.int16)

    percore = []
    ss0 = x @ v0s; sd0 = x @ v0d
    # L0 per-edge host-gathered streams (no device gather needed):
    # e0x[e] = [x0, x1, ss0(4), 0, 0] of src_e ; e0sd[e] = sd0 of dst_e
    ex_nodes = np.concatenate([x, np.zeros((1, 2), f32)], 0)
    es_nodes = np.concatenate([ss0, np.full((1, 4), SENT, f32)], 0)
    ed_nodes = np.concatenate([sd0, np.zeros((1, 4), f32)], 0)

    bnm, bna = [], []
    for L in range(2):
        gm = np.asarray(inputs[f"bn_g{L}"], f32); bt = np.asarray(inputs[f"bn_b{L}"], f32)
        mu = np.asarray(inputs[f"bn_m{L}"], f32); vr = np.asarray(inputs[f"bn_v{L}"], f32)
        m = (gm / np.sqrt(vr + BN_EPS)).astype(f32)
        bnm.append(m); bna.append((bt - mu * m).astype(f32))

    tt_col = t[:, g].astype(f32)
    tts_col = (tt_col - t0).astype(f32)
    with np.errstate(over='ignore'):
        S_col = (1.0 / (1.0 + np.exp(-(kk * (tt_col - t0 - dd)).astype(f32)))).astype(f32)

    iota = np.tile(np.arange(P, dtype=f32), (P, 1))
    sent1 = np.zeros((1, 192), f32); sent1[0, 128:132] = SENT
    sent2 = np.zeros((1, 8), f32); sent2[0, 4] = SENT

    common = dict(
        iota=iota,
        w0t=np.ascontiguousarray(W0.T),                       # (2, 512)
        w1t=np.ascontiguousarray(W1.T),                       # (128, 512)
        v1sd=np.concatenate([v1s, v1d], 1),                   # (128, 8)
        v2all=np.concatenate([W2.T, v2s, v2d], 1),            # (128, 5)
        b0col=b0.reshape(128, 1), b1col=b1.reshape(128, 1),
        b2rep=np.tile(b2.reshape(1, 3), (P, 1)),
        bnm0=bnm[0].reshape(128, 1), bna0=bna[0].reshape(128, 1),
        bnm1=bnm[1].reshape(128, 1), bna1=bna[1].reshape(128, 1),
        sent1=sent1, sent2=sent2,
    )
    for c in range(NCORES):
        aux = np.zeros((NT, P, 4), f32)
        nodes = np.arange(c * NPC, (c + 1) * NPC)
        a_flat = np.zeros((NT * P, 4), f32)
        a_flat[:NPC, 0] = tt_col[nodes]
        a_flat[:NPC, 1] = tts_col[nodes]
        a_flat[:NPC, 2] = S_col[nodes]
        a_flat[:NPC, 3] = u0
        aux[:] = a_flat.reshape(NT, P, 4)
        m = dict(common)
        m["xgidx"] = wrap16(xg[c])
        m["sdidx"] = wrap16(sg[c])
        m["dstloc"] = np.ascontiguousarray(
            dl[c].reshape(NT, nchunk, P).transpose(2, 0, 1))  # (128, NT, nchunk)
        m["aux"] = aux
        # L0 direct edge streams: local node id (or N sentinel) per edge
        srcl = np.where(xg[c] % BLK == NPC, N, (xg[c] // BLK) * NPC + xg[c] % BLK)
        dstl = (sg[c] // BLK) * NPC + sg[c] % BLK
        e0x = np.zeros((NT, nchunk, P, 8), f32)
        e0x[:, :, :, 0:2] = ex_nodes[srcl].reshape(NT, nchunk, P, 2, order='C')
        e0x[:, :, :, 2:6] = es_nodes[srcl].reshape(NT, nchunk, P, 4)
        m["e0x"] = np.ascontiguousarray(e0x.transpose(0, 2, 1, 3))  # (NT,128,nchunk,8)
        e0sd = ed_nodes[dstl].reshape(NT, nchunk, P, 4)
        m["e0sd"] = np.ascontiguousarray(e0sd.transpose(0, 2, 1, 3))
        percore.append(m)
    return percore, nchunk


# ------------------------------------------------------------- device build
def _build(nchunk):
    import concourse.bass as bass
    from concourse import bacc
    import concourse.tile as tile
    import concourse.mybir as mybir
    from concourse.masks import make_identity

    f32 = mybir.dt.float32
    i16 = mybir.dt.int16
    OP = mybir.AluOpType
    AF = mybir.ActivationFunctionType
    NIDX = nchunk * P
    W16 = NIDX // 16

    nc = bacc.Bacc("TRN2", target_bir_lowering=False, debug=False,
                   num_devices=NCORES, num_swdge_queues=4)
    I = lambda nm, sh, dt=f32: nc.dram_tensor(nm, sh, dt, kind="ExternalInput").ap()
    e0x_i = I("e0x", [NT, P, nchunk, 8])
    e0sd_i = I("e0sd", [NT, P, nchunk, 4])
    iota_i = I("iota", [P, P])
    w0t_i = I("w0t", [2, 512]); w1t_i = I("w1t", [P, 512])
    v1sd_i = I("v1sd", [P, 8]); v2all_i = I("v2all", [P, 5])
    b0c_i = I("b0col", [P, 1]); b1c_i = I("b1col", [P, 1]); b2r_i = I("b2rep", [P, 3])
    bnm0_i = I("bnm0", [P, 1]); bna0_i = I("bna0", [P, 1])
    bnm1_i = I("bnm1", [P, 1]); bna1_i = I("bna1", [P, 1])
    sent1_i = I("sent1", [1, 192]); sent2_i = I("sent2", [1, 8])
    xgidx_i = I("xgidx", [P, NT, W16], i16)
    sdidx_i = I("sdidx", [P, NT, W16], i16)
    dstloc_i = I("dstloc", [P, NT, nchunk])
    aux_i = I("aux", [NT, P, 4])
    out_o = nc.dram_tensor("out", [NPC, 2], f32, kind="ExternalOutput").ap()
    if DEBUG:
        dbg1 = nc.dram_tensor("dbg_xe1", [BLK, 192], f32, kind="ExternalOutput").ap()
        dbg2 = nc.dram_tensor("dbg_xe2", [BLK, 8], f32, kind="ExternalOutput").ap()

    with tile.TileContext(nc) as tc:
        with tc.tile_pool(name="sc", bufs=1) as sc, \
             tc.tile_pool(name="sidx", bufs=4) as sidx, \
             tc.tile_pool(name="sg", bufs=3) as sgp, \
             tc.tile_pool(name="sa", bufs=4) as sap, \
             tc.tile_pool(name="sx", bufs=3) as sxp, \
             tc.tile_pool(name="pz", bufs=2, space="PSUM") as pz, \
             tc.tile_pool(name="pden", bufs=2, space="PSUM") as pden, \
             tc.tile_pool(name="pout", bufs=2, space="PSUM") as pout, \
             tc.tile_pool(name="pmisc", bufs=2, space="PSUM") as pmisc, \
             tc.tile_pool(name="dram", bufs=1, space="DRAM") as dram:

            # ---- constants to SBUF
            def ld(ap_, sh, nm, dt=f32):
                tl = sc.tile(sh, dt, tag=nm, name=nm)
                nc.sync.dma_start(out=tl[:], in_=ap_[tuple(slice(None) for _ in sh)])
                return tl
            iota = ld(iota_i, [P, P], "c_iota"); w0t = ld(w0t_i, [2, 512], "c_w0t")
            w1t = ld(w1t_i, [P, 512], "c_w1t"); v1sd = ld(v1sd_i, [P, 8], "c_v1sd")
            v2all = ld(v2all_i, [P, 5], "c_v2all"); b0c = ld(b0c_i, [P, 1], "c_b0c")
            b1c = ld(b1c_i, [P, 1], "c_b1c"); b2r = ld(b2r_i, [P, 3], "c_b2r")
            bnm0 = ld(bnm0_i, [P, 1], "c_bnm0"); bna0 = ld(bna0_i, [P, 1], "c_bna0")
            bnm1 = ld(bnm1_i, [P, 1], "c_bnm1"); bna1 = ld(bna1_i, [P, 1], "c_bna1")
            sent1 = ld(sent1_i, [1, 192], "c_sent1"); sent2 = ld(sent2_i, [1, 8], "c_sent2")
            ident = sc.tile([P, P], f32); make_identity(nc, ident[:])
            ones = sc.tile([P, 1], f32); nc.vector.memset(ones[:], 1.0)
            x3_all = sc.tile([P, NT, 3], f32)
            aux_sb = sc.tile([P, NT, 4], f32)
            for t in range(NT):
                nc.sync.dma_start(out=aux_sb[:, t, :], in_=aux_i[t, :, :])
            uv = sc.tile([P, NT, 2], f32)

            # ---- internal DRAM
            xe1_sh = dram.tile([BLK, 192], f32)
            xe1_full = dram.tile([NROWS, 192], f32, addr_space="Shared")
            sd1_tab = dram.tile([NROWS, 64], f32)
            xe2_sh = dram.tile([BLK, 8], f32)
            xe2_full = dram.tile([NROWS, 8], f32, addr_space="Shared")
            xe2_tab = dram.tile([NROWS, 64], f32)

            nc.sync.dma_start(out=xe1_sh[NPC:NPC + 1, :], in_=sent1[:])
            nc.sync.dma_start(out=xe2_sh[NPC:NPC + 1, :], in_=sent2[:])

            def gather(tbl, idx_i, t, roww, gtag, qn):
                idx = sidx.tile([P, W16], i16, tag="idx" + gtag)
                nc.sync.dma_start(out=idx[:], in_=idx_i[:, t, :])
                gt = sgp.tile([P, nchunk, roww], f32, tag=gtag)
                nc.gpsimd.dma_gather(gt[:], tbl, idx[:], NIDX, NIDX, roww,
                                     single_packet=False, queue_num=qn)
                return gt

            def edge_aggr(t, tbl, roww, sdtbl, sdww, mlo, mhi, sslo, ssn, sdlo,
                          H, Mz, sep_denom, direct=False):
                """Returns (zT_ps (Mz,512-or-128H), den_ps or None)."""
                if direct:
                    xgt = sgp.tile([P, nchunk, roww], f32, tag="e0x", name="e0x")
                    nc.sync.dma_start(out=xgt[:], in_=e0x_i[t, :, :, :])
                    sdt = sgp.tile([P, nchunk, sdww], f32, tag="e0sd", name="e0sd")
                    nc.sync.dma_start(out=sdt[:], in_=e0sd_i[t, :, :, :])
                else:
                    xgt = gather(tbl, xgidx_i, t, roww, f"gx{roww}", (2 * t) % 4)
                    sdt = gather(sdtbl, sdidx_i, t, sdww, f"gs{sdww}", (2 * t + 1) % 4)
                lg = sxp.tile([P, nchunk, ssn], f32, tag="lg")
                nc.vector.tensor_tensor(
                    out=lg[:], in0=xgt[:, :, sslo:sslo + ssn],
                    in1=sdt[:, :, sdlo:sdlo + ssn], op=OP.add)
                pt = sxp.tile([P, nchunk, ssn], f32, tag="pt")
                nc.vector.scalar_tensor_tensor(
                    out=pt[:], in0=lg[:], scalar=0.2, in1=lg[:],
                    op0=OP.mult, op1=OP.max)
                nc.scalar.activation(out=pt[:], in_=pt[:], func=AF.Exp)
                dsl = sidx.tile([P, nchunk], f32, tag="dl")
                nc.sync.dma_start(out=dsl[:], in_=dstloc_i[:, t, :])
                zt = pz.tile([Mz, H * P], f32, space="PSUM", tag="zt")
                den = pden.tile([4, P], f32, space="PSUM", tag="den", name="den") if sep_denom else None
                for k in range(nchunk):
                    if H == 1:
                        A = sap.tile([P, P], f32, tag="A", name="A")
                        nc.vector.tensor_scalar(
                            out=A[:], in0=iota[:],
                            scalar1=dsl[:, k:k + 1], scalar2=pt[:, k, 0:1],
                            op0=OP.is_equal, op1=OP.mult)
                    else:
                        onehot = sap.tile([P, P], f32, tag="onehot")
                        nc.vector.tensor_scalar(
                            out=onehot[:], in0=iota[:], scalar1=dsl[:, k:k + 1],
                            scalar2=None, op0=OP.is_equal)
                        A = sap.tile([P, H * P], f32, tag="A", name="A")
                        for h in range(2):
                            nc.vector.tensor_scalar(
                                out=A[:, h * P:(h + 1) * P], in0=onehot[:],
                                scalar1=pt[:, k, h:h + 1], scalar2=None,
                                op0=OP.mult)
                        for h in range(2, H):
                            nc.scalar.activation(
                                out=A[:, h * P:(h + 1) * P], in_=onehot[:],
                                func=AF.Copy, scale=pt[:, k, h:h + 1])
                    nc.tensor.matmul(out=zt[:], lhsT=xgt[:, k, mlo:mhi], rhs=A[:],
                                     start=(k == 0), stop=(k == nchunk - 1))
                    if sep_denom:
                        nc.tensor.matmul(out=den[:], lhsT=pt[:, k, :],
                                         rhs=onehot[:],
                                         start=(k == 0), stop=(k == nchunk - 1))
                return zt, den

            def rdenom(dsb4, scale, bias):
                """den (4,128) sbuf -> rdT_sb (128,4) = 1/(scale*den+bias)."""
                rdp = pmisc.tile([P, 4], f32, space="PSUM", tag="misc")
                nc.tensor.transpose(out=rdp[:], in_=dsb4[:],
                                    identity=ident[0:4, 0:4])
                sc4 = sxp.tile([P, 4], f32, tag="sc4")
                nc.vector.tensor_scalar(out=sc4[:], in0=rdp[:], scalar1=scale,
                                        scalar2=bias, op0=OP.mult, op1=OP.add)
                rdt = sxp.tile([P, 4], f32, tag="rdt")
                nc.vector.reciprocal(out=rdt[:], in_=sc4[:])
                return rdt

            def epilogue01(t, zt, den, K, wt, bcol, bnmul, bnadd, vsc, nsc):
                """L0/L1 per-tile epilogue: normalize+project+bias+elu+bn+scores.
                Returns (xts_sb (c,d), scT_ps (128,nsc))."""
                zsb = sxp.tile([K if K > 2 else 2, 512], f32, tag="zsb")
                nc.vector.tensor_copy(out=zsb[:], in_=zt[:])
                dsb = sxp.tile([4, P], f32, tag="dsb")
                nc.vector.tensor_copy(out=dsb[:], in_=den[:])
                rdt = rdenom(dsb, 4.0, 4e-16)
                acc = sxp.tile([P, P], f32, tag="acc")
                for h in range(4):
                    oh = pout.tile([P, P], f32, space="PSUM", tag="oh")
                    nc.tensor.matmul(out=oh[:], lhsT=zsb[0:K, h * P:(h + 1) * P],
                                     rhs=wt[0:K, h * P:(h + 1) * P],
                                     start=True, stop=True)
                    if h == 0:
                        nc.vector.tensor_scalar(out=acc[:], in0=oh[:],
                                                scalar1=rdt[:, 0:1], scalar2=None,
                                                op0=OP.mult)
                    else:
                        nc.vector.scalar_tensor_tensor(
                            out=acc[:], in0=oh[:], scalar=rdt[:, h:h + 1],
                            in1=acc[:], op0=OP.mult, op1=OP.add)
                xtp = pmisc.tile([P, P], f32, space="PSUM", tag="misc")
                nc.tensor.transpose(out=xtp[:], in_=acc[:], identity=ident[:])
                mn = sxp.tile([P, P], f32, tag="mn")
                nc.vector.tensor_scalar(out=mn[:], in0=xtp[:], scalar1=bcol[:, :1],
                                        scalar2=0.0, op0=OP.add, op1=OP.min)
                mx = sxp.tile([P, P], f32, tag="mx")
                nc.vector.tensor_scalar(out=mx[:], in0=xtp[:], scalar1=bcol[:, :1],
                                        scalar2=0.0, op0=OP.add, op1=OP.max)
                nc.scalar.activation(out=mn[:], in_=mn[:], func=AF.Exp)
                el = sxp.tile([P, P], f32, tag="el")
                nc.vector.scalar_tensor_tensor(out=el[:], in0=mn[:], scalar=-1.0,
                                               in1=mx[:], op0=OP.add, op1=OP.add)
                xts = sxp.tile([P, P], f32, tag="xts")
                nc.vector.tensor_scalar(out=xts[:], in0=el[:], scalar1=bnmul[:, :1],
                                        scalar2=bnadd[:, :1], op0=OP.mult, op1=OP.add)
                # scores
                scp = pmisc.tile([8, P], f32, space="PSUM", tag="misc")
                nc.tensor.matmul(out=scp[0:nsc, :], lhsT=vsc[:, 0:nsc], rhs=xts[:],
                                 start=True, stop=True)
                scs = sxp.tile([8, P], f32, tag="scs")
                nc.vector.tensor_copy(out=scs[0:nsc, :], in_=scp[0:nsc, :])
                sctp = pmisc.tile([P, 8], f32, space="PSUM", tag="misc")
                nc.tensor.transpose(out=sctp[:, 0:nsc], in_=scs[0:nsc, :],
                                    identity=ident[0:nsc, 0:nsc])
                return xts, sctp

            # ================= L0 =================
            for t in range(NT):
                tl = min(P, NPC - t * P)
                zt, den = edge_aggr(t, None, 8, None, 4,
                                     0, 2, 2, 4, 0, 4, 2, True, direct=True)
                xts, sctp = epilogue01(t, zt, den, 2, w0t, b0c, bnm0, bna0,
                                       v1sd, 8)
                x1p = pmisc.tile([P, P], f32, space="PSUM", tag="misc")
                nc.tensor.transpose(out=x1p[:], in_=xts[:], identity=ident[:])
                row = sxp.tile([P, 192], f32, tag="row1")
                nc.vector.tensor_copy(out=row[:, 0:P], in_=x1p[:])
                nc.vector.tensor_copy(out=row[:, 128:136], in_=sctp[:, 0:8])
                nc.sync.dma_start(out=xe1_sh[t * P:t * P + tl, :],
                                  in_=row[0:tl, :])

            nc.gpsimd.collective_compute(
                "AllGather", OP.bypass, replica_groups=[list(range(NCORES))],
                ins=[xe1_sh.opt()], outs=[xe1_full.opt()])
            nc.sync.dma_start(out=sd1_tab[:, 0:4], in_=xe1_full[:, 132:136])

            # ================= L1 =================
            for t in range(NT):
                tl = min(P, NPC - t * P)
                zt, den = edge_aggr(t, xe1_full[:, :], 192, sd1_tab[:, :], 64,
                                    0, 128, 128, 4, 0, 4, 128, True)
                _, sctp = epilogue01(t, zt, den, 128, w1t, b1c, bnm1, bna1,
                                     v2all, 5)
                # row cols: [h2(3), 1, ss2, sd2, pad2] ; row[:,0:P] currently x2
                r2 = sxp.tile([P, 8], f32, tag="r2")
                nc.vector.tensor_copy(out=r2[:, 0:3], in_=sctp[:, 0:3])
                nc.vector.memset(r2[:, 3:4], 1.0)
                nc.vector.tensor_copy(out=r2[:, 4:6], in_=sctp[:, 3:5])
                nc.vector.memset(r2[:, 6:8], 0.0)
                nc.sync.dma_start(out=xe2_sh[t * P:t * P + tl, :], in_=r2[0:tl, :])

            nc.gpsimd.collective_compute(
                "AllGather", OP.bypass, replica_groups=[list(range(NCORES))],
                ins=[xe2_sh.opt()], outs=[xe2_full.opt()])
            nc.sync.dma_start(out=xe2_tab[:, 0:8], in_=xe2_full[:, :])

            # ================= L2 =================
            for t in range(NT):
                zt, _ = edge_aggr(t, xe2_tab[:, :], 64, xe2_tab[:, :], 64,
                                  0, 4, 4, 1, 5, 1, 4, False)
                zsb = sxp.tile([4, P], f32, tag="z2sb")
                nc.vector.tensor_copy(out=zsb[:], in_=zt[:, 0:P])
                z2tp = pmisc.tile([P, 4], f32, space="PSUM", tag="misc")
                nc.tensor.transpose(out=z2tp[:], in_=zsb[:], identity=ident[0:4, 0:4])
                dn = sxp.tile([P, 1], f32, tag="dn")
                nc.vector.tensor_scalar(out=dn[:], in0=z2tp[:, 3:4], scalar1=1e-16,
                                        scalar2=None, op0=OP.add)
                rd = sxp.tile([P, 1], f32, tag="rd")
                nc.vector.reciprocal(out=rd[:], in_=dn[:])
                y = sxp.tile([P, 3], f32, tag="y")
                nc.vector.tensor_scalar(out=y[:], in0=z2tp[:, 0:3],
                                        scalar1=rd[:, :1], scalar2=None, op0=OP.mult)
                nc.vector.tensor_tensor(out=y[:], in0=y[:], in1=b2r[:], op=OP.add)
                mn = sxp.tile([P, 3], f32, tag="mn3")
                nc.vector.tensor_scalar(out=mn[:], in0=y[:], scalar1=0.0,
                                        scalar2=None, op0=OP.min)
                mx = sxp.tile([P, 3], f32, tag="mx3")
                nc.vector.tensor_scalar(out=mx[:], in0=y[:], scalar1=0.0,
                                        scalar2=None, op0=OP.max)
                nc.scalar.activation(out=mn[:], in_=mn[:], func=AF.Exp)
                nc.vector.scalar_tensor_tensor(out=x3_all[:, t, :], in0=mn[:],
                                               scalar=-1.0, in1=mx[:],
                                               op0=OP.add, op1=OP.add)

            # ================= ODE =================
            TT_, TS_, STT = nc.vector.tensor_tensor, nc.vector.tensor_scalar, \
                nc.vector.scalar_tensor_tensor

            def tmp(tag):
                return sxp.tile([P, NT, 1], f32, tag=tag, name=tag)
            a_r = x3_all[:, :, 0:1]; gam = x3_all[:, :, 1:2]; bet = x3_all[:, :, 2:3]
            ttc = aux_sb[:, :, 0:1]; tts = aux_sb[:, :, 1:2]
            Sc = aux_sb[:, :, 2:3]; u0c = aux_sb[:, :, 3:4]

            def expneg(src_a, src_b, tag):
                o = tmp(tag)
                TT_(out=o[:], in0=src_a, in1=src_b, op=OP.mult)
                nc.scalar.activation(out=o[:], in_=o[:], func=AF.Exp, scale=-1.0)
                return o
            eb = expneg(bet, ttc, "eb"); eg = expneg(gam, ttc, "eg")
            ebs = expneg(bet, tts, "ebs"); egs = expneg(gam, tts, "egs")
            rbet = tmp("rbet"); nc.vector.reciprocal(out=rbet[:], in_=bet)
            rgam = tmp("rgam"); nc.vector.reciprocal(out=rgam[:], in_=gam)
            gmb = tmp("gmb"); TT_(out=gmb[:], in0=gam, in1=bet, op=OP.subtract)
            rgmb = tmp("rgmb"); nc.vector.reciprocal(out=rgmb[:], in_=gmb[:])
            ab = tmp("ab"); TT_(out=ab[:], in0=a_r, in1=rbet[:], op=OP.mult)
            ag = tmp("ag"); TT_(out=ag[:], in0=a_r, in1=rgam[:], op=OP.mult)
            omeb = tmp("omeb")
            TS_(out=omeb[:], in0=eb[:], scalar1=-1.0, scalar2=1.0, op0=OP.mult, op1=OP.add)
            omeg = tmp("omeg")
            TS_(out=omeg[:], in0=eg[:], scalar1=-1.0, scalar2=1.0, op0=OP.mult, op1=OP.add)
            omS = tmp("omS")
            TS_(out=omS[:], in0=Sc, scalar1=-1.0, scalar2=1.0, op0=OP.mult, op1=OP.add)
            # tilde_u = ab*(1-eb)*(1-S) + ab*S + (u0*ebs - ab)*S
            A1 = tmp("A1"); TT_(out=A1[:], in0=ab[:], in1=omeb[:], op=OP.mult)
            TT_(out=A1[:], in0=A1[:], in1=omS[:], op=OP.mult)
            B1 = tmp("B1"); TT_(out=B1[:], in0=ab[:], in1=Sc, op=OP.mult)
            C1 = tmp("C1"); TT_(out=C1[:], in0=u0c, in1=ebs[:], op=OP.mult)
            TT_(out=C1[:], in0=C1[:], in1=ab[:], op=OP.subtract)
            TT_(out=C1[:], in0=C1[:], in1=Sc, op=OP.mult)
            TT_(out=A1[:], in0=A1[:], in1=B1[:], op=OP.add)
            TT_(out=uv[:, :, 0:1], in0=A1[:], in1=C1[:], op=OP.add)
            # tilde_s
            A2 = tmp("A2"); TT_(out=A2[:], in0=ag[:], in1=omeg[:], op=OP.mult)
            D2 = tmp("D2"); TT_(out=D2[:], in0=eg[:], in1=eb[:], op=OP.subtract)
            E2 = tmp("E2"); TT_(out=E2[:], in0=a_r, in1=rgmb[:], op=OP.mult)
            TT_(out=E2[:], in0=E2[:], in1=D2[:], op=OP.mult)
            TT_(out=A2[:], in0=A2[:], in1=E2[:], op=OP.add)
            TT_(out=A2[:], in0=A2[:], in1=omS[:], op=OP.mult)
            B2 = tmp("B2"); TT_(out=B2[:], in0=ag[:], in1=Sc, op=OP.mult)
            C2 = tmp("C2"); TT_(out=C2[:], in0=bet, in1=u0c, op=OP.mult)
            TT_(out=C2[:], in0=C2[:], in1=rgmb[:], op=OP.mult)
            D3 = tmp("D3"); TT_(out=D3[:], in0=egs[:], in1=ebs[:], op=OP.subtract)
            TT_(out=C2[:], in0=C2[:], in1=D3[:], op=OP.mult)
            TT_(out=C2[:], in0=C2[:], in1=Sc, op=OP.mult)
            TT_(out=A2[:], in0=A2[:], in1=B2[:], op=OP.add)
            TT_(out=uv[:, :, 1:2], in0=A2[:], in1=C2[:], op=OP.add)

            for t in range(NT):
                tl = min(P, NPC - t * P)
                nc.sync.dma_start(out=out_o[t * P:t * P + tl, :],
                                  in_=uv[0:tl, t, :])

            if DEBUG:
                nc.sync.dma_start(out=dbg1[:, :], in_=xe1_sh[:])
                nc.sync.dma_start(out=dbg2[:, :], in_=xe2_sh[:])

    nc.compile()
    return nc


# ------------------------------------------------------------------- driver
def kernel(**inputs):
    global _last_exec_ns, _last_results
    from concourse.bass_utils import run_bass_kernel_spmd
    percore, nchunk = _prep(inputs)
    key = (nchunk, DEBUG)
    if key not in _CACHE:
        _CACHE[key] = _build(nchunk)
    nc = _CACHE[key]
    res = run_bass_kernel_spmd(nc, percore, core_ids=list(range(NCORES)),
                               trace=TRACE)
    _last_exec_ns = res.exec_time_ns
    _last_results = res
    out = np.concatenate([res.results[c]["out"] for c in range(NCORES)], 0)
    return out[:, 0].astype(np.float32), out[:, 1].astype(np.float32)
